# revision 12
# baseline (speedup 1.0000x reference)
"""GCNN rollout kernel for Trainium2 (8 NeuronCores, batch-parallel).

Math (per batch b): maintain W_k = S^k x_t for k=0..3.
  G = sum_k W_k H1[k];  Z = tanh(G);  V = Z H2;  P_k = S^k V (k=1..4)
  W_k += DT * P_{k+1};  x_{t+1} = W_0;  U_t = P_1 = (x_{t+1}-x_t)/DT.

Per-core layouts (8 batches/core, N=1024, p=q=4, f=32):
  WT   (128,1024)  partitions (k,b,p) = 32k+4b+p, free n    fp32 state
  G.T  2 psum tiles (128,1024), partitions (b,f) = 32b+f (b 0-3 / 4-7)
  Z.T  2 sbuf tiles, same layout
  V    (128,8,32)  partition = n within 128-block kb, free (kb,(b,p))
  P.T  psum (128,1024), partitions (k,b,p), free n  — 4 powers computed
       concurrently via PE column-tiling (tile_position=(0,32k)), moving
       operand (S^{k+1}).T streamed from SBUF.

Matmul precision: (a) G and (c) V in plain fp32.  (d) "bf3" mode uses a
3-round bf16 split  P = S1*V1 + S2*V1 + S1*V2  (S1=bf16(S^k.T),
S2=bf16 residual, V1=bf16(V), V2=bf16 residual): 3 cyc/row vs fp32's 4,
x64 rel err 2.8e-4 / u 1.7e-3 (numpy-validated vs reference; fp32 floor
is 6e-5 / 4.3e-4).  D_MODE="fp32" selects exact fp32 (1 round).

u_traj is reconstructed on host: U_t = (x_{t+1} - x_t)/DT exactly.
"""
import numpy as np
import ml_dtypes
import concourse.bass as bass
import concourse.mybir as mybir
from concourse.bass_utils import run_bass_kernel_spmd

FT = mybir.dt.float32
BF = mybir.dt.bfloat16

T_STEPS = 64
DT = np.float32(0.1)
N = 1024
NCORES = 8
BL = 8          # batches per core
NKB = 8         # N / 128
D_MODE = "fp32"  # "bf3" (3-round bf16 split) or "fp32" (exact, 4 cyc/row)
STUB = set()  # timing-bisection hooks (empty in production)


def _bf16(x):
    return np.asarray(x, np.float32).astype(ml_dtypes.bfloat16)


def _build_nc(repeat=1):
    nc = bass.Bass()
    SD = BF if D_MODE == "bf3" else FT
    n_srounds = 2 if D_MODE == "bf3" else 1   # S operand tensors
    S1_d = [nc.declare_dram_parameter(f"S1_{k}", [128, NKB, N], SD, isOutput=False)
            for k in range(4)]
    S2_d = [nc.declare_dram_parameter(f"S2_{k}", [128, NKB, N], SD, isOutput=False)
            for k in range(4)] if n_srounds == 2 else None
    H1_d = nc.declare_dram_parameter("H1blk", [128, 256], FT, isOutput=False)
    H2_d = nc.declare_dram_parameter("H2blk", [128, 64], FT, isOutput=False)
    x0T_d = nc.declare_dram_parameter("x0T", [32, N], FT, isOutput=False)
    V01_d = nc.declare_dram_parameter("V01", [128, NKB, 32], SD, isOutput=False)
    V02_d = (nc.declare_dram_parameter("V02", [128, NKB, 32], SD, isOutput=False)
             if D_MODE == "bf3" else None)
    xout_d = nc.declare_dram_parameter("x_out", [T_STEPS, 32, N], FT, isOutput=True)

    n_in = 4 + (4 if n_srounds == 2 else 0) + 3 + (2 if D_MODE == "bf3" else 1)

    from contextlib import ExitStack
    ctx = ExitStack()
    with ctx:
        sb = lambda name, shape, dt: ctx.enter_context(nc.sbuf_tensor(name, shape, dt))
        ps = lambda name, shape: ctx.enter_context(nc.psum_tensor(name, shape, FT))
        sem = lambda name: ctx.enter_context(nc.semaphore(name))
        S1 = [sb(f"S1_{k}_sb", [128, NKB, N], SD) for k in range(4)]
        S2 = [sb(f"S2_{k}_sb", [128, NKB, N], SD) for k in range(4)] \
            if n_srounds == 2 else None
        H1sb = sb("H1_sb", [128, 256], FT)
        H2sb = sb("H2_sb", [128, 64], FT)
        WT = sb("WT_sb", [128, N], FT)
        ZTA = sb("ZTA_sb", [128, N], FT)
        ZTB = sb("ZTB_sb", [128, N], FT)
        V1 = sb("V1_sb", [128, NKB, 32], SD)
        V2 = sb("V2_sb", [128, NKB, 32], SD) if D_MODE == "bf3" else None
        G_A = ps("G_A", [128, N])
        G_B = ps("G_B", [128, N])
        Pps = ps("Pps", [128, N])
        Vps = ps("Vps", [128, NKB, 32])
        s_in = sem("s_in")
        s_pe = sem("s_pe")
        s_act = sem("s_act")
        s_dve = sem("s_dve")
        s_out = sem("s_out")
        block = ctx.enter_context(nc.Block())

        if D_MODE == "bf3":
            rounds = [(V1, S1), (V1, S2), (V2, S1)]
        else:
            rounds = [(V1, S1)]

        def emit_d():
            """P.T[(k,b,p), n] += sum_m V[m,(b,p)] * S^{k+1}.T[m, n]."""
            last = None
            for rnd, (vt, st) in enumerate(rounds):
                for c in range(2 if "d" not in STUB else 1):
                    for kb in range(NKB if "d" not in STUB else 1):
                        for k in range(4 if "d" not in STUB else 1):
                            last = nc.tensor.matmul(
                                Pps[32 * k:32 * k + 32, c * 512:(c + 1) * 512],
                                vt[:, kb, :],
                                st[k][:, kb, c * 512:(c + 1) * 512],
                                start=(rnd == 0 and kb == 0),
                                stop=True if "d" in STUB else (rnd == len(rounds) - 1 and kb == NKB - 1),
                                tile_position=(0, 32 * k),
                                skip_group_check=True,
                            )
            return last

        n_reinit = 3 if D_MODE == "bf3" else 2
        PE_D, ACT_D, DVE_D, OUT_D = 3 * T_STEPS + 1, 2 * T_STEPS, 2 * T_STEPS + 1, 16 * T_STEPS

        @block.sync
        def _(sync):
            for k in range(4):
                sync.dma_start(S1[k][:], S1_d[k][:]).then_inc(s_in, 16)
            if n_srounds == 2:
                for k in range(4):
                    sync.dma_start(S2[k][:], S2_d[k][:]).then_inc(s_in, 16)
            sync.dma_start(H1sb[:], H1_d[:]).then_inc(s_in, 16)
            sync.dma_start(H2sb[:], H2_d[:]).then_inc(s_in, 16)
            for r in range(repeat):
                if r:
                    sync.wait_ge(s_out, OUT_D * r)       # prior x-DMAs done
                    sync.wait_ge(s_dve, DVE_D * r)       # prior rollout done
                sync.dma_start(WT[0:32, :], x0T_d[:]).then_inc(s_in, 16)
                sync.dma_start(V1[:], V01_d[:]).then_inc(s_in, 16)
                if D_MODE == "bf3":
                    sync.dma_start(V2[:], V02_d[:]).then_inc(s_in, 16)
                for t in range(T_STEPS):
                    sync.wait_ge(s_dve, DVE_D * r + 2 * t + 3)
                    sync.dma_start(xout_d[t], WT[0:32, :]).then_inc(s_out, 16)
            sync.wait_ge(s_out, OUT_D * repeat)

        @block.tensor
        def _(pe):
          for r in range(repeat):
            PB, AB, DB = PE_D * r, ACT_D * r, DVE_D * r
            pe.wait_ge(s_in, 16 * (n_in + n_reinit * r))
            if r:
                pe.wait_ge(s_dve, DB)            # prev rollout fully consumed Pps
            emit_d().then_inc(s_pe, 1)           # P = S^{k+1} x0  -> PB+1
            for t in range(T_STEPS):
                # (a) G.T = H1blk.T @ W.T  (fp32)
                pe.wait_ge(s_dve, DB + 2 * t + 1)  # W state ready
                last = None
                for half, G in (((0, G_A), (1, G_B)) if "a" not in STUB else ((0, G_A),)):
                    for c in range(2 if "a" not in STUB else 1):
                        last = nc.tensor.matmul(
                            G[:, c * 512:(c + 1) * 512],
                            H1sb[:, half * 128:(half + 1) * 128],
                            WT[:, c * 512:(c + 1) * 512],
                            start=True, stop=True,
                        )
                last.then_inc(s_pe, 1)           # -> PB+3t+2
                # (c) V[mb] = ZTA[:,mb].T @ H2A + ZTB[:,mb].T @ H2B  (fp32)
                pe.wait_ge(s_act, AB + 2 * t + 2)
                for mb in range(NKB if "c" not in STUB else 1):
                    if "c" not in STUB:
                        nc.tensor.matmul(
                            Vps[:, mb, :], ZTA[:, mb * 128:(mb + 1) * 128],
                            H2sb[:, 0:32], start=True, stop=False)
                    last = nc.tensor.matmul(
                        Vps[:, mb, :], ZTB[:, mb * 128:(mb + 1) * 128],
                        H2sb[:, 32:64], start=("c" in STUB), stop=True)
                last.then_inc(s_pe, 1)           # -> PB+3t+3
                # (d) P = S^{k+1} V
                pe.wait_ge(s_dve, DB + 2 * t + 2)  # V-prep done
                emit_d().then_inc(s_pe, 1)       # -> PB+3t+4

        @block.scalar
        def _(act):
          for r in range(repeat):
            PB = PE_D * r
            for t in range(T_STEPS):
                act.wait_ge(s_pe, PB + 3 * t + 2)
                fn = (mybir.ActivationFunctionType.Tanh if "tanh" not in STUB
                      else mybir.ActivationFunctionType.Copy)
                nc.scalar.activation(ZTA[:], G_A[:], fn).then_inc(s_act, 1)
                nc.scalar.activation(ZTB[:], G_B[:], fn).then_inc(s_act, 1)

        @block.vector
        def _(dve):
          for r in range(repeat):
            PB, OB = PE_D * r, OUT_D * r
            dve.wait_ge(s_pe, PB + 1)
            nc.vector.tensor_copy(WT[32:64, :], Pps[0:32, :])
            nc.vector.tensor_copy(WT[64:96, :], Pps[32:64, :])
            nc.vector.tensor_copy(WT[96:128, :], Pps[64:96, :]).then_inc(s_dve, 1)
            for t in range(T_STEPS):
                dve.wait_ge(s_pe, PB + 3 * t + 3)  # V in psum
                if D_MODE == "bf3":
                    # V1 = bf16(V), V2 = bf16(V - V1)
                    nc.vector.tensor_copy(V1[:], Vps[:])
                    nc.vector.tensor_sub(V2[:], Vps[:], V1[:]).then_inc(s_dve, 1)
                else:
                    nc.vector.tensor_copy(V1[:], Vps[:]).then_inc(s_dve, 1)
                dve.wait_ge(s_pe, PB + 3 * t + 4)  # P ready
                if t or r:
                    dve.wait_ge(s_out, OB + 16 * t)  # x-DMA of t-1 completed (WAR)
                if "e" in STUB:
                    nc.vector.tensor_copy(WT[0:32, :], Pps[0:32, :]).then_inc(s_dve, 1)
                else:
                    nc.vector.scalar_tensor_tensor(
                        WT[:], Pps[:], float(DT), WT[:],
                        mybir.AluOpType.mult, mybir.AluOpType.add,
                    ).then_inc(s_dve, 1)         # -> 2t+3

    return nc


def _host_prep(x0, S, H1, H2):
    S = np.asarray(S, np.float32)
    H1 = np.asarray(H1, np.float32)
    H2 = np.asarray(H2, np.float32)
    x0 = np.asarray(x0, np.float32)

    pows = [S]
    for _ in range(3):
        pows.append((pows[-1] @ S).astype(np.float32))
    # (S^{k+1}).T in (partition, kb, n) layout
    SkT = [np.ascontiguousarray(p.T).reshape(NKB, 128, N).transpose(1, 0, 2)
           for p in pows]
    if D_MODE == "bf3":
        S1 = [np.ascontiguousarray(_bf16(s)) for s in SkT]
        S2 = [np.ascontiguousarray(_bf16(s - s1.astype(np.float32)))
              for s, s1 in zip(SkT, S1)]
    else:
        S1 = [np.ascontiguousarray(s) for s in SkT]
        S2 = None

    H1blk = np.zeros((128, 256), np.float32)
    for k in range(4):
        for b in range(8):
            for p in range(4):
                H1blk[32 * k + 4 * b + p, 32 * b:32 * b + 32] = H1[k, p, :]
    # H2blk: col block [0:32] pairs with Z-tile A (batches 0-3), [32:64] with
    # tile B (batches 4-7); within a block cols are (b,q) over all 8 batches.
    H2blk = np.zeros((128, 64), np.float32)
    for b in range(4):
        for f in range(32):
            H2blk[32 * b + f, 4 * b:4 * b + 4] = H2[0, f, :]
    for b in range(4, 8):
        for f in range(32):
            H2blk[32 * (b - 4) + f, 32 + 4 * b:32 + 4 * b + 4] = H2[0, f, :]

    per_core = []
    for c in range(NCORES):
        x0c = x0[c * BL:(c + 1) * BL]                  # (8, 1024, 4)
        x0T = np.ascontiguousarray(
            x0c.transpose(0, 2, 1).reshape(32, N))     # (32, 1024)
        V0 = np.ascontiguousarray(
            x0c.reshape(BL, NKB, 128, 4).transpose(2, 1, 0, 3).reshape(128, NKB, 32))
        m = {"x0T": x0T, "H1blk": H1blk, "H2blk": H2blk}
        if D_MODE == "bf3":
            V01 = _bf16(V0)
            m["V01"] = np.ascontiguousarray(V01)
            m["V02"] = np.ascontiguousarray(_bf16(V0 - V01.astype(np.float32)))
        else:
            m["V01"] = V0
        for k in range(4):
            m[f"S1_{k}"] = S1[k]
            if S2 is not None:
                m[f"S2_{k}"] = S2[k]
        per_core.append(m)
    return per_core


def run_on_hw(x0, S, H1, H2, trace=False, repeat=1, nc=None):
    in_maps = _host_prep(x0, S, H1, H2)
    if nc is None:
        nc = _build_nc(repeat)
    return run_bass_kernel_spmd(nc, in_maps, list(range(NCORES)), trace=trace)


def kernel(x0, S, H1, H2):
    x0 = np.asarray(x0, np.float32)
    res = run_on_hw(x0, S, H1, H2)
    x_traj = np.empty((64, T_STEPS + 1, N, 4), np.float32)
    x_traj[:, 0] = x0
    for c in range(NCORES):
        xo = np.asarray(res.results[c]["x_out"])       # (64, 32, 1024)
        x_traj[c * BL:(c + 1) * BL, 1:] = (
            xo.reshape(T_STEPS, BL, 4, N).transpose(1, 0, 3, 2))
    u_traj = ((x_traj[:, 1:].astype(np.float64)
               - x_traj[:, :-1].astype(np.float64)) / float(DT)).astype(np.float32)
    return x_traj, u_traj
